# revision 1
# baseline (speedup 1.0000x reference)
"""Trainium2 Bass kernel for nn_DBGNN (2-layer hetero SAGEConv GNN).

The network is linear up to the final softmax, so the 128-dim feature
chain is folded on the host into small per-node aggregate vectors:

  layer0:  s_* = segmean of xa (9 cols: [x(8), 1]) over each of the 4
           edge directions; only [s, x, 1] per node is kept.
  layer1:  u_t0/u_t2 = segmean over (c->t)/(p->t) of za_c/za_p rows
           (za = [s(9), x(8), 1] padded into 32-col tables).
  out:     softmax(u_t0 @ G0 + u_t2 @ G2 + za_t @ Gt + g_const)

All folded matrices (G*) are built on the host in float64.

Device mapping (8 NeuronCores, dst-sharded):
  - Phase A payloads (xa[src] rows, scaled by 1/max(cnt,1) of their dst)
    are pre-gathered on the host, grouped by 128-row dst windows with
    per-window counts padded to the max across cores (SPMD-uniform
    program structure).  Scatter = one-hot matmul on PE: onehot built
    by DVE is_equal against a host-uploaded iota tile per 128-edge tile.
  - Phase A writes ZC/ZP (row-major, gathered later) and ST0T/ST2T
    (transposed, read densely in phase C).
  - ZC/ZP shards are AllGather'ed across the 8 cores.
  - Phase B gathers ZCfull/ZPfull rows per edge with [P,1] indirect
    DMAs (128 rows/instr), scales by per-edge 1/cnt, and scatter-matmuls
    into transposed u-windows kept in SBUF.
  - Phase C: 5 accumulated matmuls per 128-row window -> logitsT [10,128],
    + g_const, exp on ACT, PE transpose, batched row softmax, DMA out.
"""
import numpy as np

# ---- problem sizes (hardcoded; kernel.py must be self-contained) ----
NC, NT, NP = 100000, 300000, 50000
F8, DIMC, D, OUT = 8, 16, 128, 10
EM, EI = 300000, 600000
NCORES = 8
NCs, NTs, NPs = NC // NCORES, NT // NCORES, NP // NCORES    # 12500, 37500, 6250
PW = 128                                                     # dst window rows
WC, WT, WP = -(-NCs // PW), -(-NTs // PW), -(-NPs // PW)     # 98, 293, 49
NCsp, NTsp, NPsp = WC * PW, WT * PW, WP * PW                 # padded shard rows
ZCOLS = 32            # z-table width (cols: 0:9 s | 16:24 x | 24 one)
TBL_C, TBL_P = NCsp * NCORES, NPsp * NCORES                  # full table rows


def _fold(W_col, b_col, Wn, Wr, b_lin, W_out, b_out):
    dt = np.float64
    W_col, b_col = np.asarray(W_col, dt), np.asarray(b_col, dt)
    Wn, Wr, b_lin = np.asarray(Wn, dt), np.asarray(Wr, dt), np.asarray(b_lin, dt)
    W_out, b_out = np.asarray(W_out, dt), np.asarray(b_out, dt)
    P = np.zeros((3, 8, D), dt)
    c = np.zeros((3, D), dt)
    for s in range(3):
        for f in range(8):
            P[s, f, f * 16:(f + 1) * 16] = W_col[s, f]
            c[s, f * 16:(f + 1) * 16] = b_col[s, f]
    Pa = [np.vstack([P[s], c[s]]) for s in range(3)]  # [9,128] each
    Mc = np.vstack([Pa[1] @ Wn[0, 1], P[0] @ Wr[0, 1],
                    (c[0] @ Wr[0, 1] + b_lin[0, 1])[None]])              # [18,128]
    Mp = np.vstack([Pa[1] @ Wn[0, 3], P[2] @ Wr[0, 3],
                    (c[2] @ Wr[0, 3] + b_lin[0, 3])[None]])              # [18,128]
    Mt = np.vstack([Pa[0] @ (.5 * Wn[0, 0]), Pa[2] @ (.5 * Wn[0, 2]),
                    P[1] @ (.5 * (Wr[0, 0] + Wr[0, 2])),
                    (c[1] @ (.5 * (Wr[0, 0] + Wr[0, 2])) + .5 * (b_lin[0, 0] + b_lin[0, 2]))[None]])  # [27,128]
    G0 = Mc @ (.5 * Wn[1, 0]) @ W_out                                    # [18,10]
    G2 = Mp @ (.5 * Wn[1, 2]) @ W_out                                    # [18,10]
    Gt = Mt @ (.5 * (Wr[1, 0] + Wr[1, 2])) @ W_out                       # [27,10]
    gc = (.5 * (b_lin[1, 0] + b_lin[1, 2])) @ W_out + b_out              # [10]

    def z32(G18):   # za order [s(9), x(8), 1] -> 32-col table order
        Gd = np.zeros((32, OUT), np.float32)
        Gd[0:9] = G18[0:9]
        Gd[16:24] = G18[9:17]
        Gd[24] = G18[17]
        return Gd
    G0d, G2d = z32(G0), z32(G2)
    Gt0 = np.zeros((16, OUT), np.float32); Gt0[0:9] = Gt[0:9]
    Gt2 = np.zeros((16, OUT), np.float32); Gt2[0:9] = Gt[9:18]
    Gtx = np.zeros((16, OUT), np.float32); Gtx[0:9] = Gt[18:27]
    return G0d, G2d, Gt0, Gt2, Gtx, gc.astype(np.float32)


class DirStruct:
    """Shared (cross-core) structure for one scatter direction."""
    def __init__(self, n_windows, pad, name):
        self.name = name
        self.n_windows = n_windows
        self.pad = pad                       # [W] padded per-window counts
        self.start = np.zeros(n_windows + 1, np.int64)
        np.cumsum(pad, out=self.start[1:])
        self.S = int(self.start[-1])
        self.T = -(-self.S // 128)           # tiles
        # tile -> primary window (window containing slot 128*t)
        self.w1 = np.searchsorted(self.start, np.arange(self.T) * 128, "right") - 1
        # window -> list of (tile, k) with k = w - w1(t) in {0,1}
        self.win_tiles = []
        for w in range(n_windows):
            t0, t1 = self.start[w] // 128, (self.start[w + 1] - 1) // 128
            lst = []
            for t in range(int(t0), int(t1) + 1):
                k = w - int(self.w1[t])
                assert 0 <= k <= 1, (name, w, t, k)
                lst.append((t, k))
            self.win_tiles.append(lst)


def _build_dir(dst_all, core_of_dst, dst_local, n_windows, name):
    """Bin edges of one direction by (core, window); pad counts to max8.
    Returns (struct, per-core edge->slot assignment lists)."""
    W = n_windows
    win = dst_local // PW
    cnt = np.zeros((NCORES, W), np.int64)
    np.add.at(cnt, (core_of_dst, win), 1)
    pad = np.maximum(cnt.max(axis=0), 1)
    st = DirStruct(W, pad, name)
    # per-core: order edges by window; slots[e] = position in padded stream
    slots = []
    for k in range(NCORES):
        m = np.nonzero(core_of_dst == k)[0]
        order = np.argsort(win[m], kind="stable")
        me = m[order]
        wsorted = win[me]
        # position within window = running index
        within = np.arange(len(me)) - np.searchsorted(wsorted, wsorted)
        slot = st.start[wsorted] + within
        slots.append((me, slot))
    return st, slots


def _preprocess(inp):
    x_c = np.asarray(inp["x_c"], np.float32)
    x_t = np.asarray(inp["x_t"], np.float32)
    x_p = np.asarray(inp["x_p"], np.float32)
    ems = np.asarray(inp["e_makes_src"], np.int64)
    emd = np.asarray(inp["e_makes_dst"], np.int64)
    eis = np.asarray(inp["e_in_src"], np.int64)
    eid = np.asarray(inp["e_in_dst"], np.int64)
    G0d, G2d, Gt0, Gt2, Gtx, gc = _fold(inp["W_col"], inp["b_col"], inp["Wn"],
                                        inp["Wr"], inp["b_lin"], inp["W_out"], inp["b_out"])

    def xa(x):
        n = x.shape[0]
        a = np.zeros((n, 16), np.float32)
        a[:, 0:8] = x
        a[:, 8] = 1.0
        return a
    xa_c, xa_t, xa_p = xa(x_c), xa(x_t), xa(x_p)

    # global counts -> inv per dst node (for payload prescale)
    def inv_cnt(dst, n):
        c = np.bincount(dst, minlength=n).astype(np.float32)
        return 1.0 / np.maximum(c, 1.0)
    inv_t0 = inv_cnt(emd, NT)
    inv_c = inv_cnt(ems, NC)
    inv_t2 = inv_cnt(eid, NT)
    inv_p = inv_cnt(eis, NP)

    # directions: (dst ids, dst shard size, windows, src xa table, inv table)
    dirs = {}
    specs = [
        ("t0", emd, NTs, WT, xa_c, ems, inv_t0),
        ("c",  ems, NCs, WC, xa_t, emd, inv_c),
        ("t2", eid, NTs, WT, xa_p, eis, inv_t2),
        ("p",  eis, NPs, WP, xa_t, eid, inv_p),
    ]
    for name, dst, shard, W, xs, src, invd in specs:
        core = dst // shard
        dloc = dst - core * shard
        st, slots = _build_dir(dst, core, dloc, W, name)
        # per-core arrays
        pays, drels = [], []
        for k in range(NCORES):
            me, slot = slots[k]
            pay = np.zeros((st.T * 128, 16), np.float32)
            drel = np.full((st.T * 128,), -1.0, np.float32)
            pay[slot] = xs[src[me]] * invd[dst[me]][:, None]
            dl = dloc[me]
            drel[slot] = dl - (st.w1[slot // 128] * PW)
            pays.append(pay.reshape(st.T, 128, 16))
            # dstrel as [128, T]: slot s -> partition s%128, col s//128
            drels.append(np.ascontiguousarray(drel.reshape(st.T, 128).T))
        dirs[name] = dict(st=st, pay=pays, drel=drels, slots=slots)

    # phase-B gather data for t0/t2 (same structure as phase-A t0/t2)
    for name, src, dst, invd, shard_src, nsp in (
            ("t0", ems, emd, inv_t0, NCs, NCsp), ("t2", eis, eid, inv_t2, NPs, NPsp)):
        st = dirs[name]["st"]
        offs, invs = [], []
        for k in range(NCORES):
            me, slot = dirs[name]["slots"][k]
            off = np.zeros((st.T * 128,), np.int32)
            iv = np.zeros((st.T * 128,), np.float32)
            sc = src[me] // shard_src          # source core
            srow = sc * nsp + (src[me] - sc * shard_src)   # padded full-table row
            off[slot] = srow.astype(np.int32)
            iv[slot] = invd[dst[me]]
            offs.append(np.ascontiguousarray(off.reshape(st.T, 128).T))
            invs.append(np.ascontiguousarray(iv.reshape(st.T, 128).T))
        dirs[name]["offs"] = offs
        dirs[name]["invs"] = invs

    # per-core static tables
    iota0 = np.tile(np.arange(128, dtype=np.float32), (128, 1))
    iota1 = iota0 + 128.0
    ident = np.eye(10, dtype=np.float32)
    gccol = gc.reshape(10, 1)

    def zinit(xa_full, shard, nsp, k):
        zi = np.zeros((nsp, ZCOLS), np.float32)
        zi[0:shard, 16:24] = xa_full[k * shard:(k + 1) * shard, 0:8]
        zi[0:shard, 24] = 1.0
        return zi

    def xatT(k):
        a = np.zeros((16, NTsp), np.float32)
        a[0:8, 0:NTs] = x_t[k * NTs:(k + 1) * NTs].T
        a[8, 0:NTs] = 1.0
        return a

    in_maps = []
    for k in range(NCORES):
        m = dict(
            payA_t0=dirs["t0"]["pay"][k], drel_t0=dirs["t0"]["drel"][k],
            payA_c=dirs["c"]["pay"][k],  drel_c=dirs["c"]["drel"][k],
            payA_t2=dirs["t2"]["pay"][k], drel_t2=dirs["t2"]["drel"][k],
            payA_p=dirs["p"]["pay"][k],  drel_p=dirs["p"]["drel"][k],
            offs_t0=dirs["t0"]["offs"][k], invs_t0=dirs["t0"]["invs"][k],
            offs_t2=dirs["t2"]["offs"][k], invs_t2=dirs["t2"]["invs"][k],
            zc_init=zinit(xa_c, NCs, NCsp, k),
            zp_init=zinit(xa_p, NPs, NPsp, k),
            xatT=xatT(k),
            iota0=iota0, iota1=iota1, ident=ident,
            G0d=G0d, G2d=G2d, Gt0=Gt0, Gt2=Gt2, Gtx=Gtx, gccol=gccol,
        )
        in_maps.append(m)
    structs = {n: dirs[n]["st"] for n in dirs}
    return in_maps, structs


# ======================= device program =======================

def _build_nc(structs):
    import concourse.bacc as bacc
    import concourse.bass as bass
    import concourse.mybir as mybir
    import concourse.tile as tile

    st_t0, st_c, st_t2, st_p = structs["t0"], structs["c"], structs["t2"], structs["p"]
    nc = bacc.Bacc("TRN2", debug=False)
    f32, i32 = mybir.dt.float32, mybir.dt.int32
    AG = "AllGather"
    BYP = mybir.AluOpType.bypass
    ISEQ = mybir.AluOpType.is_equal
    MUL = mybir.AluOpType.mult
    ADD = mybir.AluOpType.add

    # ---- dram I/O ----
    def din(name, shape, dt=f32):
        return nc.dram_tensor(name, shape, dt, kind="ExternalInput")
    payA = {n: din(f"payA_{n}", [structs[n].T, 128, 16]) for n in ("t0", "c", "t2", "p")}
    drel = {n: din(f"drel_{n}", [128, structs[n].T]) for n in ("t0", "c", "t2", "p")}
    offs = {n: din(f"offs_{n}", [128, structs[n].T], i32) for n in ("t0", "t2")}
    invs = {n: din(f"invs_{n}", [128, structs[n].T]) for n in ("t0", "t2")}
    zc_init = din("zc_init", [NCsp, ZCOLS])
    zp_init = din("zp_init", [NPsp, ZCOLS])
    xatT = din("xatT", [16, NTsp])
    iota0 = din("iota0", [128, 128]); iota1 = din("iota1", [128, 128])
    ident = din("ident", [10, 10])
    G0d = din("G0d", [32, OUT]); G2d = din("G2d", [32, OUT])
    Gt0 = din("Gt0", [16, OUT]); Gt2 = din("Gt2", [16, OUT]); Gtx = din("Gtx", [16, OUT])
    gccol = din("gccol", [10, 1])
    outp = nc.dram_tensor("outp", [NTsp, OUT], f32, kind="ExternalOutput")
    # internal DRAM
    ZC = nc.dram_tensor("ZC", [NCsp, ZCOLS], f32)
    ZP = nc.dram_tensor("ZP", [NPsp, ZCOLS], f32)
    ZCfull = nc.dram_tensor("ZCfull", [TBL_C, ZCOLS], f32, addr_space="Shared")
    ZPfull = nc.dram_tensor("ZPfull", [TBL_P, ZCOLS], f32, addr_space="Shared")
    ST0T = nc.dram_tensor("ST0T", [16, NTsp], f32)
    ST2T = nc.dram_tensor("ST2T", [16, NTsp], f32)
    UW0 = nc.dram_tensor("UW0", [32, NTsp], f32)
    UW2 = nc.dram_tensor("UW2", [32, NTsp], f32)

    RG = [list(range(NCORES))]
    _tn = [0]

    def _nm(tag):
        _tn[0] += 1
        return f"{tag}_{_tn[0]}"

    with tile.TileContext(nc, num_cores=NCORES) as tc:
        with (
            tc.tile_pool(name="const", bufs=1) as constp,
            tc.tile_pool(name="stream", bufs=2) as streamp,
            tc.tile_pool(name="work", bufs=4) as workp,
            tc.tile_pool(name="zt", bufs=6) as ztp,
            tc.tile_pool(name="stage", bufs=2) as stagep,
            tc.tile_pool(name="ps", bufs=3, space="PSUM") as psp,
            tc.tile_pool(name="psc", bufs=4, space="PSUM") as pscp,
        ):
            # ---- constants to SBUF ----
            iota_sb = [constp.tile([128, 128], f32, tag="iota0", name=_nm("iota0")),
                       constp.tile([128, 128], f32, tag="iota1", name=_nm("iota1"))]
            nc.sync.dma_start(iota_sb[0][:], iota0[:])
            nc.sync.dma_start(iota_sb[1][:], iota1[:])
            ident_sb = constp.tile([10, 10], f32, tag="ident", name=_nm("ident"))
            nc.sync.dma_start(ident_sb[:], ident[:])
            gc_sb = constp.tile([10, 1], f32, tag="gc", name=_nm("gc"))
            nc.sync.dma_start(gc_sb[:], gccol[:])
            gmat_sb = {}
            for nm, h, k in (("G0d", G0d, 32), ("G2d", G2d, 32),
                             ("Gt0", Gt0, 16), ("Gt2", Gt2, 16), ("Gtx", Gtx, 16)):
                gmat_sb[nm] = constp.tile([k, OUT], f32, tag=nm, name=_nm("t"))
                nc.sync.dma_start(gmat_sb[nm][:], h[:])
            drel_sb = {}
            for n in ("t0", "c", "t2", "p"):
                drel_sb[n] = constp.tile([128, structs[n].T], f32, tag=f"drel{n}", name=_nm("t"))
                nc.sync.dma_start(drel_sb[n][:], drel[n][:])
            offs_sb, invs_sb = {}, {}
            for n in ("t0", "t2"):
                offs_sb[n] = constp.tile([128, structs[n].T], i32, tag=f"offs{n}", name=_nm("t"))
                nc.sync.dma_start(offs_sb[n][:], offs[n][:])
                invs_sb[n] = constp.tile([128, structs[n].T], f32, tag=f"invs{n}", name=_nm("t"))
                nc.sync.dma_start(invs_sb[n][:], invs[n][:])

            # ---- init ZC/ZP from host tables (bounce through SBUF) ----
            for zini, ztab, w in ((zc_init, ZC, WC), (zp_init, ZP, WP)):
                half = -(-w // 2)
                for h0 in range(0, w, half):
                    hn = min(half, w - h0)
                    bt = stagep.tile([128, half, ZCOLS], f32, tag="zinit", name=_nm("zinit"))
                    nc.sync.dma_start(
                        bt[:, 0:hn, :],
                        zini[:].rearrange("(w p) c -> p w c", p=128)[:, h0:h0 + hn, :])
                    nc.sync.dma_start(
                        ztab[:].rearrange("(w p) c -> p w c", p=128)[:, h0:h0 + hn, :],
                        bt[:, 0:hn, :])

            # ---- phase A scatter for one direction ----
            def phase_a(n, out_rowmajor, ztab=None, stT=None, w_table=None):
                s = structs[n]
                NB = 32
                nw = s.n_windows
                # stage buffers
                if out_rowmajor:
                    NSW = 14 if nw % 14 == 0 else 7
                else:
                    NSW = 16
                pay_b = None
                stage_t = None
                sw = 0  # windows in current stage
                w0 = 0  # first window of current stage
                for w in range(nw):
                    ps = psp.tile([128, 16] if out_rowmajor else [16, 128], f32, tag="ps", name=_nm("ps"))
                    tiles = s.win_tiles[w]
                    for j, (t, k) in enumerate(tiles):
                        # payload batch load on demand
                        bidx = t // NB
                        if pay_b is None or bidx != pay_b[0]:
                            nb = min(NB, s.T - bidx * NB)
                            pb = streamp.tile([128, NB, 16], f32, tag=f"payA{n}", name=_nm("t"))
                            nc.sync.dma_start(
                                pb[:, 0:nb, :],
                                payA[n][bidx * NB: bidx * NB + nb].rearrange("a p c -> p a c"))
                            pay_b = (bidx, pb)
                        oh = workp.tile([128, 128], f32, tag="oh", name=_nm("oh"))
                        nc.vector.tensor_scalar(
                            out=oh[:], in0=iota_sb[k][:],
                            scalar1=drel_sb[n][:, t:t + 1], scalar2=None, op0=ISEQ)
                        pay_ap = pay_b[1][:, t - pay_b[0] * NB, :]
                        if out_rowmajor:
                            nc.tensor.matmul(ps[:], lhsT=oh[:], rhs=pay_ap,
                                             start=(j == 0), stop=(j == len(tiles) - 1))
                        else:
                            nc.tensor.matmul(ps[:], lhsT=pay_ap, rhs=oh[:],
                                             start=(j == 0), stop=(j == len(tiles) - 1))
                    # flush window to stage
                    if stage_t is None:
                        stage_t = stagep.tile(
                            [128, NSW, 16] if out_rowmajor else [16, NSW * 128],
                            f32, tag=f"stA{n}", name=_nm("t"))
                        w0, sw = w, 0
                    if out_rowmajor:
                        nc.vector.tensor_copy(out=stage_t[:, sw, :], in_=ps[:])
                    else:
                        nc.vector.tensor_copy(out=stage_t[:, sw * 128:(sw + 1) * 128], in_=ps[:])
                    sw += 1
                    if sw == NSW or w == nw - 1:
                        if out_rowmajor:
                            nc.sync.dma_start(
                                ztab[:].rearrange("(w p) c -> p w c", p=128)[:, w0:w0 + sw, 0:16],
                                stage_t[:, 0:sw, :])
                        else:
                            nc.sync.dma_start(stT[:, w0 * 128:(w0 + sw) * 128],
                                              stage_t[:, 0:sw * 128])
                        stage_t = None

            # ---- phase B scatter (indirect gather + matmul) into u buffer ----
            def phase_b(n, table, utab):
                s = structs[n]
                gathered = {}
                NSW = 16
                stage_t = None
                w0 = sw = 0
                for w in range(s.n_windows):
                    ps = psp.tile([32, 128], f32, tag="ps", name=_nm("ps"))
                    tiles = s.win_tiles[w]
                    for j, (t, k) in enumerate(tiles):
                        if t not in gathered:
                            zt = ztp.tile([128, ZCOLS], f32, tag="ztile", name=_nm("ztile"))
                            nc.gpsimd.indirect_dma_start(
                                out=zt[:], out_offset=None, in_=table[:],
                                in_offset=bass.IndirectOffsetOnAxis(
                                    ap=offs_sb[n][:, t:t + 1], axis=0))
                            zs = ztp.tile([128, ZCOLS], f32, tag="zscaled", name=_nm("zscaled"))
                            nc.vector.tensor_scalar_mul(zs[:], zt[:],
                                                        invs_sb[n][:, t:t + 1])
                            gathered = {t: zs}  # keep only latest (straddle uses t again next window)
                        zs = gathered[t]
                        oh = workp.tile([128, 128], f32, tag="oh", name=_nm("oh"))
                        nc.vector.tensor_scalar(
                            out=oh[:], in0=iota_sb[k][:],
                            scalar1=drel_sb[n][:, t:t + 1], scalar2=None, op0=ISEQ)
                        nc.tensor.matmul(ps[:], lhsT=zs[:], rhs=oh[:],
                                         start=(j == 0), stop=(j == len(tiles) - 1))
                    if stage_t is None:
                        stage_t = stagep.tile([32, NSW * 128], f32, tag="stB", name=_nm("stB"))
                        w0, sw = w, 0
                    nc.vector.tensor_copy(out=stage_t[:, sw * 128:(sw + 1) * 128], in_=ps[:])
                    sw += 1
                    if sw == NSW or w == s.n_windows - 1:
                        nc.sync.dma_start(utab[:, w0 * 128:(w0 + sw) * 128],
                                          stage_t[:, 0:sw * 128])
                        stage_t = None

            # ---- emit program ----
            phase_a("c", True, ztab=ZC)
            nc.gpsimd.collective_compute(AG, BYP, replica_groups=RG,
                                         ins=[ZC[:]], outs=[ZCfull[:]])
            phase_a("p", True, ztab=ZP)
            nc.gpsimd.collective_compute(AG, BYP, replica_groups=RG,
                                         ins=[ZP[:]], outs=[ZPfull[:]])
            phase_a("t0", False, stT=ST0T)
            phase_a("t2", False, stT=ST2T)
            phase_b("t0", ZCfull, UW0)
            phase_b("t2", ZPfull, UW2)

            # ---- phase C ----
            NB3 = 8
            nbatches = -(-WT // NB3)
            soft_t = None
            for b in range(nbatches):
                w0 = b * NB3
                nbw = min(NB3, WT - w0)
                uw0_b = streamp.tile([32, NB3 * 128], f32, tag="uw0b", name=_nm("uw0b"))
                uw2_b = streamp.tile([32, NB3 * 128], f32, tag="uw2b", name=_nm("uw2b"))
                st0_b = streamp.tile([16, NB3 * 128], f32, tag="st0b", name=_nm("st0b"))
                st2_b = streamp.tile([16, NB3 * 128], f32, tag="st2b", name=_nm("st2b"))
                xat_b = streamp.tile([16, NB3 * 128], f32, tag="xatb", name=_nm("xatb"))
                nc.sync.dma_start(uw0_b[:, 0:nbw * 128], UW0[:, w0 * 128:(w0 + nbw) * 128])
                nc.sync.dma_start(uw2_b[:, 0:nbw * 128], UW2[:, w0 * 128:(w0 + nbw) * 128])
                nc.sync.dma_start(st0_b[:, 0:nbw * 128], ST0T[:, w0 * 128:(w0 + nbw) * 128])
                nc.sync.dma_start(st2_b[:, 0:nbw * 128], ST2T[:, w0 * 128:(w0 + nbw) * 128])
                nc.sync.dma_start(xat_b[:, 0:nbw * 128], xatT[:, w0 * 128:(w0 + nbw) * 128])
                soft_t = stagep.tile([128, NB3, OUT], f32, tag="soft", name=_nm("soft"))
                for i in range(nbw):
                    w = w0 + i
                    sl = slice(w * 128, (w + 1) * 128)
                    sli = slice(i * 128, (i + 1) * 128)
                    ps = pscp.tile([10, 128], f32, tag="psC", name=_nm("psC"))
                    nc.tensor.matmul(ps[:], lhsT=gmat_sb["G0d"][:], rhs=uw0_b[:, sli], start=True, stop=False)
                    nc.tensor.matmul(ps[:], lhsT=gmat_sb["G2d"][:], rhs=uw2_b[:, sli], start=False, stop=False)
                    nc.tensor.matmul(ps[:], lhsT=gmat_sb["Gt0"][:], rhs=st0_b[:, sli], start=False, stop=False)
                    nc.tensor.matmul(ps[:], lhsT=gmat_sb["Gt2"][:], rhs=st2_b[:, sli], start=False, stop=False)
                    nc.tensor.matmul(ps[:], lhsT=gmat_sb["Gtx"][:], rhs=xat_b[:, sli], start=False, stop=True)
                    logT = workp.tile([10, 128], f32, tag="logT", name=_nm("logT"))
                    nc.vector.tensor_scalar_add(logT[:], ps[:], gc_sb[:, 0:1])
                    expT = workp.tile([10, 128], f32, tag="expT", name=_nm("expT"))
                    nc.scalar.activation(expT[:], logT[:],
                                         mybir.ActivationFunctionType.Exp)
                    ps2 = pscp.tile([128, 10], f32, tag="psC", name=_nm("psC"))
                    nc.tensor.transpose(ps2[:], expT[:], ident_sb[:])
                    nc.vector.tensor_copy(out=soft_t[:, i, :], in_=ps2[:])
                sums = workp.tile([128, NB3], f32, tag="sums", name=_nm("sums"))
                nc.vector.tensor_reduce(out=sums[:, 0:nbw], in_=soft_t[:, 0:nbw, :],
                                        axis=mybir.AxisListType.X, op=ADD)
                rec = workp.tile([128, NB3], f32, tag="rec", name=_nm("rec"))
                nc.vector.reciprocal(rec[:, 0:nbw], sums[:, 0:nbw])
                nc.vector.tensor_tensor(
                    out=soft_t[:, 0:nbw, :], in0=soft_t[:, 0:nbw, :],
                    in1=rec[:, 0:nbw].to_broadcast([128, nbw, OUT]), op=MUL)
                nc.sync.dma_start(
                    outp[:].rearrange("(w p) c -> p w c", p=128)[:, w0:w0 + nbw, :],
                    soft_t[:, 0:nbw, :])

    nc.compile()
    return nc


# ======================= runner =======================

class _Runner:
    def __init__(self, nc, n_cores=NCORES):
        import jax
        import concourse.mybir as mybir
        from concourse import bass2jax
        from jax.sharding import Mesh, PartitionSpec
        from jax.experimental.shard_map import shard_map
        bass2jax.install_neuronx_cc_hook()
        self.jax = jax
        self.n_cores = n_cores
        partition_name = nc.partition_id_tensor.name if nc.partition_id_tensor else None
        in_names, out_names, out_avals, zero_outs = [], [], [], []
        for alloc in nc.m.functions[0].allocations:
            if not isinstance(alloc, mybir.MemoryLocationSet):
                continue
            name = alloc.memorylocations[0].name
            if alloc.kind == "ExternalInput":
                if name != partition_name:
                    in_names.append(name)
            elif alloc.kind == "ExternalOutput":
                out_names.append(name)
                shape = tuple(alloc.tensor_shape)
                dtype = mybir.dt.np(alloc.dtype)
                out_avals.append(jax.core.ShapedArray(shape, dtype))
                zero_outs.append(np.zeros(shape, dtype))
        assert nc.dbg_addr is None
        self.in_names, self.out_names, self.out_avals = in_names, out_names, out_avals
        self.zero_outs = zero_outs
        n_params = len(in_names)
        self.n_params = n_params
        all_names = in_names + out_names + ([partition_name] if partition_name else [])

        def _body(*args):
            operands = list(args)
            if partition_name is not None:
                operands.append(bass2jax.partition_id_tensor())
            return tuple(bass2jax._bass_exec_p.bind(
                *operands, out_avals=tuple(out_avals), in_names=tuple(all_names),
                out_names=tuple(out_names), lowering_input_output_aliases=(),
                sim_require_finite=True, sim_require_nnan=True, nc=nc))

        devices = jax.devices()[:n_cores]
        mesh = Mesh(np.asarray(devices), ("core",))
        in_specs = (PartitionSpec("core"),) * (n_params + len(out_names))
        out_specs = (PartitionSpec("core"),) * len(out_names)
        self._fn = jax.jit(
            shard_map(_body, mesh=mesh, in_specs=in_specs, out_specs=out_specs,
                      check_rep=False), keep_unused=True)

    def prepare(self, in_maps):
        concat = [np.concatenate([np.asarray(m[n]) for m in in_maps], axis=0)
                  for n in self.in_names]
        zeros = [np.zeros((self.n_cores * z.shape[0], *z.shape[1:]), z.dtype)
                 for z in self.zero_outs]
        self._args = [self.jax.device_put(a) for a in concat + zeros]
        self.jax.block_until_ready(self._args)

    def run(self):
        outs = self._fn(*self._args)
        outs = [np.asarray(o) for o in outs]
        return [
            {n: outs[i].reshape(self.n_cores, *self.out_avals[i].shape)[c]
             for i, n in enumerate(self.out_names)}
            for c in range(self.n_cores)
        ]

    def time_burst(self, burst=8, reps=4):
        import time
        totals = []
        for _ in range(reps):
            t0 = time.perf_counter_ns()
            outs = [self._fn(*self._args) for _ in range(burst)]
            self.jax.block_until_ready(outs)
            totals.append(time.perf_counter_ns() - t0)
            del outs
        return min(totals), totals


_CACHE = {}


def _get_runner(inp):
    in_maps, structs = _preprocess(inp)
    key = tuple((n, structs[n].S) for n in sorted(structs))
    if key not in _CACHE:
        nc = _build_nc(structs)
        _CACHE[key] = _Runner(nc)
    r = _CACHE[key]
    r.prepare(in_maps)
    return r


def kernel(**inputs) -> np.ndarray:
    r = _get_runner(inputs)
    res = r.run()
    out = np.empty((NT, OUT), np.float32)
    for k in range(NCORES):
        out[k * NTs:(k + 1) * NTs] = res[k]["outp"][0:NTs]
    return out

